# revision 28
# baseline (speedup 1.0000x reference)
"""Trainium2 Bass kernel for LocalSelfAttention2d — v2 (engine-balanced).

Full inputs in, full outputs out. Data-parallel over batch B=16 across 8
NeuronCores (2 images per core). Weights/position table replicated.

Design (per image; spatial layouts window-major m = win*64+ph*8+pw):
  A. x raster f32 DMA'd in (chunk-interleaved); GPSIMD scatter-cast -> x_wm
     bf16 window-major (Pool has no PSUM port, so it gets the SBUF-only job).
  B. q/k projection, w stationary, x_wm moving -> PSUM win-major; fat evac.
  C. v projection transposed: x_wm window-pair block stationary, w_v moving
     -> vt[j(2 windows), 256 vchan]; fat evac.
  D. attention over 32 global (head, 16-window-block) units, software-
     pipelined with lag 3: per step scores(u), filler, denom(u-2), o(u-3).
     exp (ACT), *exp(bias) (DVE), and the fused recip*mult custom DVE op
     hide behind PE work of later units. B/C units of the next image and
     E units of the previous image are the fillers.
  E. output projection; epilogue on ACT (Identity + per-channel bias +
     win->raster scatter); y stores batched [128,2048] and issued from SP
     (idle once all x loads are done; the last group splits in half so
     its first store starts two units earlier).

Engine busy per core (cost model): PE ~123.5us (82% duty), DVE ~107us,
ACT ~99us, Pool ~20us; 149.8us total vs 510.6us baseline.
"""
import os
import numpy as np

B, C, H, W = 16, 256, 64, 64
P, HEADS, D = 8, 8, 32
NCORES = 8
B_LOC = B // NCORES  # 2
HW = H * W  # 4096
NW = 8  # windows per row/col

PPROJ_BUFS = int(os.environ.get("KV2_PPROJ", "3"))
PSCORE_BUFS = int(os.environ.get("KV2_PSCORE", "2"))
PDEN_BUFS = int(os.environ.get("KV2_PDEN", "1"))
MERGE_SD = os.environ.get("KV2_MERGESD", "0") == "1"  # sc+dn share a 3-ring
FILL_POS = int(os.environ.get("KV2_FILLPOS", "1"))  # 0=end 1=mid 2=predo
NORM_ENG = os.environ.get("KV2_NORM", "dve")  # pool|dve
POBANK_BUFS = int(os.environ.get("KV2_POBANK", "2"))
Y_VIA_ACT = os.environ.get("KV2_YACT", "0") == "1"
EBIAS_ENG = os.environ.get("KV2_EBIAS", "dve")  # dve|pool
FILL_PER_STEP = int(os.environ.get("KV2_FILL", "1"))
FILL_RAMP = int(os.environ.get("KV2_FILLRAMP", "0"))  # 0=off else ramp step
LAG_D = int(os.environ.get("KV2_LAG", "3"))
# qk evac engine per mc chunk (q0,q1,k0,k1): 'a'=ACT, 'v'=DVE
QK_EVAC = os.environ.get("KV2_QKEVAC", "aava")
# o-bank evac engines for (par0, par1): 'a'=ACT, 'v'=DVE
O_EVAC = os.environ.get("KV2_OEVAC", "av")

FUSE_NORM = os.environ.get("KV2_FUSE", "1") == "1"
V_EVAC = os.environ.get("KV2_VEVAC", "dve")  # dve|mix

_CACHE = {}


def _register_recip_mul():
    """Register a fused custom-DVE op: out = approx(1/in0) * in1.

    Same BITWISE_NOT exponent-flip seed + ONE Newton-Raphson pass as
    reciprocal_approx_fast (which uses two), then multiplies by Src1 —
    six pipeline stages, one DVE instruction instead of recip + mult.
    ~0.4% worst-case reciprocal error; fine for softmax denominators."""
    from concourse import dve_ops
    from concourse.dve_spec import (
        AluOp, Bin, Spec, Src0, Src1, C0, C1, lower, _has_src1,
    )
    from concourse.dve_uop import DveOpSpec

    for op in dve_ops.OPS:
        if op.name == "RECIP1NR_MUL_ANT":
            return op

    def _ref(in0, in1, s0, s1, imm2):
        not_x = (~in0.view(np.int32)).view(np.float32)
        y0 = not_x * s0
        y1 = y0 * (s1 - in0 * y0)
        return (y1 * in1).astype(np.float32)

    _not_x = Bin(AluOp.BITWISE_NOT, Src0, Src0)
    _y0 = _not_x * C0
    _y1 = _y0 * (C1 - Src0 * _y0)
    op = dve_ops.DveOp(
        "RECIP1NR_MUL_ANT",
        Spec(body=_y1 * Src1, reference=_ref),
        subdim=False,
        uops_sha={},
    )
    dve_ops.OPS.append(op)
    dve_ops.CUSTOM_DVE_SPECS[op.name] = op.spec
    dve_ops._SUB_OPCODE_FOR_NAME[op.name] = (
        max(dve_ops._SUB_OPCODE_FOR_NAME.values()) + 1)
    assert dve_ops._SUB_OPCODE_FOR_NAME[op.name] < 0x20
    # pin the lowered-ucode sha so DveOp.compile's drift check passes
    for ver in ("v3", "v4"):
        try:
            s = DveOpSpec(
                name=op.name,
                opcode=dve_ops.get_dve_sub_opcode(op.name),
                uops=lower(op.spec, ver=ver),
                rd1_en=_has_src1(op.spec),
            )
            op.uops_sha[ver] = s.sha(ver)
        except Exception:
            pass
    return op


def _rel_bias_np(position):
    coords = np.stack(
        np.meshgrid(np.arange(P), np.arange(P), indexing="ij"), -1
    ).reshape(P * P, 2)
    rel = coords[None, :, :] - coords[:, None, :] + P
    return position[:, rel[..., 0], rel[..., 1]]  # [heads, i(query), j(key)]


def _build():
    import concourse.bass as bass  # noqa: F401
    import concourse.tile as tile
    from concourse import bacc, mybir

    f32 = mybir.dt.float32
    bf16 = mybir.dt.bfloat16
    MULT = mybir.AluOpType.mult
    DIV = mybir.AluOpType.divide
    EXP = mybir.ActivationFunctionType.Exp
    IDENT = mybir.ActivationFunctionType.Identity

    recip_mul_op = _register_recip_mul()
    nc = bacc.Bacc("TRN2", target_bir_lowering=False, debug=False,
                   num_devices=NCORES)

    x_d = nc.dram_tensor("x_sh", [B_LOC, C, HW], bf16,
                         kind="ExternalInput").ap()
    # all bf16 constants packed into one [128, 6272] tensor (one DMA):
    # cols 0:1024 wqk(2x512) | 1024:1536 wv(2x256) | 1536:2048 wo(2x256)
    # | 2048:6144 ebias(8x512) | 6144:6272 ones_blk
    cpk_d = nc.dram_tensor("consts_pk", [128, 6272], bf16,
                           kind="ExternalInput").ap()
    bout_d = nc.dram_tensor("b_out2", [2, 128, 1], f32, kind="ExternalInput").ap()
    y_d = nc.dram_tensor("y_sh", [B_LOC, C, HW], bf16,
                         kind="ExternalOutput").ap()

    from contextlib import ExitStack
    with tile.TileContext(nc) as tc:
        with ExitStack() as stack:
            constp = stack.enter_context(tc.tile_pool(name="const", bufs=1))
            xinp = stack.enter_context(tc.tile_pool(name="xin", bufs=2))
            xwmp = stack.enter_context(tc.tile_pool(name="xwm", bufs=32))
            qkp = stack.enter_context(tc.tile_pool(name="qkwm", bufs=8))
            vtp = stack.enter_context(tc.tile_pool(name="vt", bufs=2))
            attp = stack.enter_context(tc.tile_pool(name="at", bufs=int(os.environ.get("KV2_ATB", "5"))))
            at2p = stack.enter_context(tc.tile_pool(name="at2", bufs=int(os.environ.get("KV2_AT2B", "5"))))
            rrp = (None if FUSE_NORM else
                   stack.enter_context(tc.tile_pool(name="rr", bufs=2)))
            osbp = stack.enter_context(tc.tile_pool(name="osb", bufs=4))
            ytp = stack.enter_context(tc.tile_pool(name="yt", bufs=4))
            pprojp = stack.enter_context(tc.tile_pool(
                name="pproj", bufs=1 if MERGE_SD else PPROJ_BUFS,
                space="PSUM"))
            if MERGE_SD:
                psdp = stack.enter_context(tc.tile_pool(
                    name="psd", bufs=3, space="PSUM"))
                pscorep = pdenp = psdp
            else:
                pscorep = stack.enter_context(tc.tile_pool(
                    name="pscore", bufs=PSCORE_BUFS, space="PSUM"))
                pdenp = stack.enter_context(tc.tile_pool(
                    name="pden", bufs=PDEN_BUFS, space="PSUM"))
                psdp = None
            pobankp = stack.enter_context(tc.tile_pool(
                name="pobank", bufs=POBANK_BUFS, space="PSUM"))
            # ---- constants (weights DMA'd first; ebias+ones can land
            # later — they are first read in phase D) ----
            cpk = constp.tile([128, 6272], bf16, tag="cpk", name="cpk")
            wqk_sb = [cpk[:, kc * 512:(kc + 1) * 512] for kc in range(2)]
            wv_sb = [cpk[:, 1024 + kc * 256:1024 + (kc + 1) * 256]
                     for kc in range(2)]
            wo_sb = [cpk[:, 1536 + kc * 256:1536 + (kc + 1) * 256]
                     for kc in range(2)]
            ebias_sb = [cpk[:, 2048 + h * 512:2048 + (h + 1) * 512]
                        for h in range(HEADS)]
            ones_sb = cpk[:, 6144:6272]
            bo = constp.tile([128, 2], f32, tag="bo", name="bo")
            nc.sync.dma_start(
                bo[:].rearrange("p (kc one) -> p kc one", kc=2, one=1),
                bout_d[:, :, :].rearrange("kc p one -> p kc one"))
            bout_sb = [bo[:, kc:kc + 1] for kc in range(2)]
            # warm the ACT function table while the first x DMAs run
            warm = constp.tile([128, 1], f32, tag="warm", name="warm")
            nc.scalar.copy(warm[:], bout_sb[0][:])

            def phase_A(b, after_first=None):
                """x load + scatter; returns x_wm[cc][wr] per-window-row
                tiles [128, 512] so consumers unblock as rows arrive.
                after_first is emitted right after the first DMA so small
                const loads don't delay the first x transfer (or vice
                versa)."""
                x_wm = [[xwmp.tile([128, 512], bf16, tag="xwm", name="xwm")
                         for _ in range(NW)] for _ in range(2)]
                first = [True]
                for t2 in range(2):
                    for cc in range(2):  # cc inner: both chunks arrive early
                        xi = xinp.tile([128, 2048], bf16, tag="xin",
                                       name="xin")
                        base = x_d[b, cc * 128:(cc + 1) * 128,
                                   t2 * 2048:(t2 + 1) * 2048]
                        if b == 0 and t2 == 0:
                            # image 0 startup: land window-row 0 first so
                            # the scatter + projections begin ~1.5us sooner
                            nc.sync.dma_start(xi[:, :512], base[:, :512])
                            nc.sync.dma_start(xi[:, 512:], base[:, 512:])
                        else:
                            nc.sync.dma_start(xi[:], base)
                        if first[0]:
                            first[0] = False
                            if after_first:
                                after_first()
                        for u in range(4):
                            wr = 4 * t2 + u
                            src = xi[:, u * 512:(u + 1) * 512].rearrange(
                                "p (ph ww pw) -> p ph ww pw", ph=P, ww=NW, pw=P
                            )
                            dst = x_wm[cc][wr][:].rearrange(
                                "p (ww ph pw) -> p ph ww pw", ww=NW, ph=P, pw=P
                            )
                            # image 0's scatter paces startup: split it
                            # across Pool and DVE; image 1 overlaps compute
                            if b == 0 and u % 2 == 1:
                                nc.vector.tensor_copy(dst, src)
                            else:
                                nc.gpsimd.tensor_copy(dst, src)
                return x_wm

            def make_BC(x_wm):
                """Thunks for q/k/v projection units (B and C interleaved)."""
                q_wm = [qkp.tile([128, HW], bf16, tag="qkwm", name="qwm")
                        for _ in range(2)]
                k_wm = [qkp.tile([128, HW], bf16, tag="qkwm", name="kwm")
                        for _ in range(2)]
                vt = vtp.tile([128, 32 * 256], bf16, tag="vt", name="vt")
                dests = [q_wm[0], q_wm[1], k_wm[0], k_wm[1]]

                def b_unit(mc, nt):
                    def run():
                        ps = pprojp.tile([128, 512], f32, tag="pp", name="pp")
                        for kc in range(2):
                            nc.tensor.matmul(
                                ps[:],
                                lhsT=wqk_sb[kc][:, mc * 128:(mc + 1) * 128],
                                rhs=x_wm[kc][nt][:],
                                start=(kc == 0), stop=(kc == 1),
                            )
                        dst = dests[mc][:, nt * 512:(nt + 1) * 512]
                        if QK_EVAC[mc] == 'a':
                            nc.scalar.copy(dst, ps[:])
                        else:
                            nc.vector.tensor_copy(dst, ps[:])
                    return run

                def c_unit(pe):
                    def run():
                        ps = pprojp.tile([128, 512], f32, tag="pp", name="pp")
                        for pp_ in range(2):
                            p = pe + pp_
                            wr, wc = p // 4, p % 4
                            for kc in range(2):
                                nc.tensor.matmul(
                                    ps[:, pp_ * 256:(pp_ + 1) * 256],
                                    lhsT=x_wm[kc][wr][:, wc * 128:
                                                      (wc + 1) * 128],
                                    rhs=wv_sb[kc][:],
                                    start=(kc == 0), stop=(kc == 1),
                                )
                        dst = vt[:, pe * 256:(pe + 2) * 256]
                        if V_EVAC == "mix" and (pe // 2) % 2:
                            nc.scalar.copy(dst, ps[:])
                        else:
                            nc.vector.tensor_copy(dst, ps[:])
                    return run

                units = []
                # nt-major so the window-rows phase D needs first are
                # produced first; C pairs of the matching window rows ride
                # along (pairs 4nt..4nt+3 live in window-row nt)
                cs = [c_unit(pe) for pe in range(0, 32, 2)]
                bs = [b_unit(mc, nt) for nt in range(8) for mc in range(4)]
                ci = iter(cs)
                for i, bu in enumerate(bs):
                    units.append(bu)
                    if i % 2 == 1:
                        nxt = next(ci, None)
                        if nxt:
                            units.append(nxt)
                units.extend(ci)
                return q_wm, k_wm, vt, units

            def phase_D(qkv, fillers, e_factory=None):
                """Attention for one image; pulls a filler thunk between
                pipeline steps. e_factory(o_sb) returns 16 output-projection
                thunks for THIS image (mc-major); the 4 units of each
                16-window block are emitted right after that block's o
                evacuation."""
                q_wm, k_wm, vt = qkv
                o_sb = [osbp.tile([128, HW], bf16, tag="osb", name="osb")
                        for _ in range(2)]
                e_units = e_factory(o_sb) if e_factory else None
                fit = iter(fillers)

                step_no = [0]

                def fill():
                    n = FILL_PER_STEP
                    if FILL_RAMP and step_no[0] >= FILL_RAMP:
                        n += 1
                    step_no[0] += 1
                    for _ in range(n):
                        f = next(fit, None)
                        if f:
                            f()

                NU = 4 * HEADS  # global units u = blk*8 + h
                obanks = [{} for _ in range(4)]
                at_t = [None] * NU
                at2_t = [None] * NU

                def get_obank(blk, hc, par):
                    # lazy alloc: first o matmul of the block allocates,
                    # giving the previous block's evacs time to free bufs
                    ob = obanks[blk]
                    if (hc, par) not in ob:
                        ob[(hc, par)] = pobankp.tile(
                            [128, 512], f32, tag="ob", name="ob")
                    return ob[(hc, par)]

                def do_scores(u):
                    blk, h = u // 8, u % 8
                    hc, hr = h // 4, h % 4
                    ps_s = pscorep.tile([128, 512], f32, tag="sc", name="sc")
                    for wl in range(16):
                        w = blk * 16 + wl
                        par, pl = w % 2, wl // 2
                        nc.tensor.matmul(
                            ps_s[par * 64:(par + 1) * 64,
                                 pl * 64:(pl + 1) * 64],
                            lhsT=k_wm[hc][hr * 32:(hr + 1) * 32,
                                          w * 64:(w + 1) * 64],
                            rhs=q_wm[hc][hr * 32:(hr + 1) * 32,
                                         w * 64:(w + 1) * 64],
                            start=True, stop=True,
                            tile_position=(hr * 32, par * 64),
                        )
                    at = attp.tile([128, 512], bf16, tag="at", name="at")
                    nc.scalar.activation(at[:], ps_s[:], EXP)
                    if EBIAS_ENG == "pool":
                        nc.gpsimd.tensor_tensor(
                            at[:], at[:], ebias_sb[h], MULT)
                    else:
                        nc.vector.tensor_tensor(
                            at[:], at[:], ebias_sb[h], MULT)
                    at_t[u] = at

                def do_denom(u):
                    h = u % 8
                    ps_d = pdenp.tile([128, 512], f32, tag="dn", name="dn")
                    nc.tensor.matmul(ps_d[:], lhsT=ones_sb[:],
                                     rhs=at_t[u][:], start=True, stop=True)
                    at2 = at2p.tile([128, 512], bf16, tag="at2", name="at2")
                    if FUSE_NORM:
                        from concourse.dve_ops import RECIP_APPROX_FAST_CONSTS
                        c = RECIP_APPROX_FAST_CONSTS
                        nc.vector._custom_dve(
                            recip_mul_op, out=at2[:], in0=ps_d[:],
                            in1=at_t[u][:], s0=c["s0"], s1=c["s1"])
                    else:
                        rr = rrp.tile([128, 512], f32, tag="rr", name="rr")
                        nc.vector.reciprocal_approx_fast(rr[:], ps_d[:])
                        eng = NORM_ENG if NORM_ENG != "split" else (
                            "pool" if h % 2 else "dve")
                        if eng == "pool":
                            nc.gpsimd.tensor_tensor(at2[:], at_t[u][:],
                                                    rr[:], MULT)
                        else:
                            nc.vector.tensor_tensor(at2[:], at_t[u][:],
                                                    rr[:], MULT)
                    at2_t[u] = at2
                    at_t[u] = None

                def do_o(u):
                    blk, h = u // 8, u % 8
                    hc, hr = h // 4, h % 4
                    for wl in range(16):
                        w = blk * 16 + wl
                        par, pl = w % 2, wl // 2
                        pglob = blk * 8 + pl
                        nc.tensor.matmul(
                            get_obank(blk, hc, par)[hr * 32:(hr + 1) * 32,
                                                    pl * 64:(pl + 1) * 64],
                            lhsT=vt[par * 64:(par + 1) * 64,
                                    pglob * 256 + h * 32:
                                    pglob * 256 + (h + 1) * 32],
                            rhs=at2_t[u][par * 64:(par + 1) * 64,
                                         pl * 64:(pl + 1) * 64],
                            start=True, stop=True,
                            tile_position=(par * 64, hr * 32),
                        )
                    at2_t[u] = None
                    if u % 8 == 7:
                        finish_blk(blk)

                def finish_blk(blk):
                    for hc in range(2):
                        for par in range(2):
                            dst = o_sb[hc][:, blk * 1024:(blk + 1) * 1024]
                            dv = dst.rearrange("p (pl par i) -> p par pl i",
                                               pl=8, par=2, i=64)[:, par]
                            if O_EVAC[par] == 'v':
                                nc.vector.tensor_copy(
                                    dv, obanks[blk][(hc, par)][:])
                            else:
                                nc.scalar.copy(dv, obanks[blk][(hc, par)][:])
                    if e_units is not None:
                        for mc in range(2):
                            for ng in range(2):
                                e_units[mc * 8 + 2 * blk + ng]()

                for step in range(NU + LAG_D):
                    if step < NU:
                        do_scores(step)
                    if FILL_POS == 1:
                        fill()
                    if LAG_D - 1 <= step < NU + LAG_D - 1:
                        do_denom(step - (LAG_D - 1))
                    if FILL_POS == 2:
                        fill()
                    if step >= LAG_D:
                        do_o(step - LAG_D)
                    if FILL_POS == 0:
                        fill()
                # drain leftover fillers
                for f in fit:
                    f()
                return o_sb

            def make_E(b, o_sb):
                """Thunks for output projection units; y DMA'd in [128,2048]
                batches of 4 nt units to amortize HWDGE issue cost."""
                yts = {}

                def e_unit(mc, nt):
                    def run():
                        ps = pprojp.tile([128, 512], f32, tag="pp", name="pp")
                        for kc in range(2):
                            nc.tensor.matmul(
                                ps[:],
                                lhsT=wo_sb[kc][:, mc * 128:(mc + 1) * 128],
                                rhs=o_sb[kc][:, nt * 512:(nt + 1) * 512],
                                start=(kc == 0), stop=(kc == 1),
                            )
                        g = nt // 4
                        if (mc, g) not in yts:
                            yts[(mc, g)] = ytp.tile([128, 2048], bf16,
                                                    tag="yt", name="yt")
                        yt = yts[(mc, g)]
                        sl = yt[:, (nt % 4) * 512:(nt % 4 + 1) * 512]
                        ydst = sl.rearrange(
                            "p (ph ww pw) -> p ww ph pw", ph=P, ww=NW, pw=P)
                        nc.scalar.activation(
                            ydst, ps[:].rearrange(
                                "p (ww ph pw) -> p ww ph pw",
                                ww=NW, ph=P, pw=P),
                            IDENT, bias=bout_sb[mc][:])
                        if g == 1:
                            # tail: store each 512-slice as soon as its
                            # epilogue lands so the final store is small
                            q0 = (nt % 4) * 512
                            ydma = y_d[b, mc * 128:(mc + 1) * 128,
                                       g * 2048 + q0:g * 2048 + q0 + 512]
                            (nc.scalar if Y_VIA_ACT else
                             nc.sync).dma_start(ydma, yt[:, q0:q0 + 512])
                        elif nt % 4 == 3:
                            ydma = y_d[b, mc * 128:(mc + 1) * 128,
                                       g * 2048:(g + 1) * 2048]
                            (nc.scalar if Y_VIA_ACT else
                             nc.sync).dma_start(ydma, yt[:])
                    return run
                return [e_unit(mc, nt) for mc in range(2) for nt in range(8)]

            # ---- emission schedule ----
            x_wm0 = phase_A(0, after_first=lambda: nc.sync.dma_start(
                cpk[:, :2048], cpk_d[:, :2048]))
            nc.sync.dma_start(cpk[:, 2048:], cpk_d[:, 2048:])
            q0, k0, vt0, bc0 = make_BC(x_wm0)
            # emit enough of B/C(0) to cover phase D block 0, then feed the
            # rest (plus all of B/C(1)) into D(0)'s step loop as fillers
            for u in bc0:
                u()
            x_wm1 = phase_A(1)
            q1, k1, vt1, bc1 = make_BC(x_wm1)
            if os.environ.get("KV2_E0FILL", "1") == "1":
                o_sb0 = phase_D((q0, k0, vt0), bc1)
                e0 = make_E(0, o_sb0)
                o_sb1 = phase_D((q1, k1, vt1), e0,
                                e_factory=lambda o: make_E(1, o))
            else:
                o_sb0 = phase_D((q0, k0, vt0), bc1,
                                e_factory=lambda o: make_E(0, o))
                o_sb1 = phase_D((q1, k1, vt1), [],
                                e_factory=lambda o: make_E(1, o))

    nc.compile()
    return nc


def _prep_consts(w_proj, position, w_out, b_out):
    import ml_dtypes
    bf16 = ml_dtypes.bfloat16
    scale = 1.0 / np.sqrt(np.float32(D))
    w_qkT = np.ascontiguousarray(w_proj[:512].T).astype(np.float32)
    w_qkT[:, :256] *= scale
    w_qkT = w_qkT.astype(bf16)
    w_vT = np.ascontiguousarray(w_proj[512:].T).astype(bf16)
    w_outT = np.ascontiguousarray(w_out.T).astype(bf16)
    bias = _rel_bias_np(np.asarray(position, np.float32))  # [h, i, j]
    eb = np.exp(bias).astype(np.float32)
    # ebias[h][rows j | j, cols 8 x (64 i)] = exp(bias[h, i, j])
    ebt = np.transpose(eb, (0, 2, 1))  # [h, j, i]
    ebias = np.empty((HEADS, 128, 512), np.float32)
    for h in range(HEADS):
        ebias[h] = np.tile(ebt[h], (2, 8))
    ebias = ebias.astype(bf16)
    ones_blk = np.zeros((128, 128), np.float32)
    ones_blk[:64, :64] = 1.0
    ones_blk[64:, 64:] = 1.0
    ones_blk = ones_blk.astype(bf16)
    # pack all bf16 consts: wqk | wv | wo | ebias | ones  (see _build)
    cpk = np.empty((128, 6272), bf16)
    cpk[:, 0:512] = w_qkT[:128]
    cpk[:, 512:1024] = w_qkT[128:]
    cpk[:, 1024:1280] = w_vT[:128]
    cpk[:, 1280:1536] = w_vT[128:]
    cpk[:, 1536:1792] = w_outT[:128]
    cpk[:, 1792:2048] = w_outT[128:]
    for h in range(HEADS):
        cpk[:, 2048 + h * 512:2048 + (h + 1) * 512] = ebias[h]
    cpk[:, 6144:6272] = ones_blk
    b_out2 = np.ascontiguousarray(
        np.asarray(b_out, np.float32).reshape(2, 128, 1))
    return {
        "consts_pk": cpk,
        "b_out2": b_out2,
    }


def kernel(x, w_proj, position, w_out, b_out):
    from concourse.bass_utils import run_bass_kernel_spmd

    if "nc" not in _CACHE:
        _CACHE["nc"] = _build()
    nc = _CACHE["nc"]

    import ml_dtypes
    consts = _prep_consts(w_proj, position, w_out, b_out)
    x = np.asarray(x, np.float32).reshape(B, C, HW).astype(ml_dtypes.bfloat16)
    in_maps = []
    for i in range(NCORES):
        m = dict(consts)
        m["x_sh"] = np.ascontiguousarray(x[i * B_LOC:(i + 1) * B_LOC])
        in_maps.append(m)

    res = run_bass_kernel_spmd(nc, in_maps, core_ids=list(range(NCORES)))
    out = np.concatenate([res.results[i]["y_sh"] for i in range(NCORES)],
                         axis=0)
    return out.reshape(B, C, H, W).astype(np.float32)



# revision 29
# speedup vs baseline: 1.0177x; 1.0177x over previous
"""Trainium2 Bass kernel for LocalSelfAttention2d — v2 (engine-balanced).

Full inputs in, full outputs out. Data-parallel over batch B=16 across 8
NeuronCores (2 images per core). Weights/position table replicated.

Design (per image; spatial layouts window-major m = win*64+ph*8+pw):
  A. x raster f32 DMA'd in (chunk-interleaved); GPSIMD scatter-cast -> x_wm
     bf16 window-major (Pool has no PSUM port, so it gets the SBUF-only job).
  B. q/k projection, w stationary, x_wm moving -> PSUM win-major; fat evac.
  C. v projection transposed: x_wm window-pair block stationary, w_v moving
     -> vt[j(2 windows), 256 vchan]; fat evac.
  D. attention over 32 global (head, 16-window-block) units, software-
     pipelined with lag 3: per step scores(u), filler, denom(u-2), o(u-3).
     exp (ACT), *exp(bias) (DVE), and the fused recip*mult custom DVE op
     hide behind PE work of later units. B/C units of the next image and
     E units of the previous image are the fillers.
  E. output projection; epilogue on ACT (Identity + per-channel bias +
     win->raster scatter); y stores batched [128,2048] and issued from SP
     (idle once all x loads are done; the last group splits in half so
     its first store starts two units earlier).

Engine busy per core (cost model): PE ~123.5us (82% duty), DVE ~107us,
ACT ~99us, Pool ~20us; 149.8us total vs 510.6us baseline.
"""
import os
import numpy as np

B, C, H, W = 16, 256, 64, 64
P, HEADS, D = 8, 8, 32
NCORES = 8
B_LOC = B // NCORES  # 2
HW = H * W  # 4096
NW = 8  # windows per row/col

PPROJ_BUFS = int(os.environ.get("KV2_PPROJ", "3"))
PSCORE_BUFS = int(os.environ.get("KV2_PSCORE", "2"))
PDEN_BUFS = int(os.environ.get("KV2_PDEN", "1"))
MERGE_SD = os.environ.get("KV2_MERGESD", "0") == "1"  # sc+dn share a 3-ring
FILL_POS = int(os.environ.get("KV2_FILLPOS", "1"))  # 0=end 1=mid 2=predo
NORM_ENG = os.environ.get("KV2_NORM", "dve")  # pool|dve
POBANK_BUFS = int(os.environ.get("KV2_POBANK", "2"))
Y_VIA_ACT = os.environ.get("KV2_YACT", "0") == "1"
EBIAS_ENG = os.environ.get("KV2_EBIAS", "dve")  # dve|pool
FILL_PER_STEP = int(os.environ.get("KV2_FILL", "1"))
FILL_RAMP = int(os.environ.get("KV2_FILLRAMP", "0"))  # 0=off else ramp step
LAG_D = int(os.environ.get("KV2_LAG", "3"))
# qk evac engine per mc chunk (q0,q1,k0,k1): 'a'=ACT, 'v'=DVE
QK_EVAC = os.environ.get("KV2_QKEVAC", "aava")
# o-bank evac engines for (par0, par1): 'a'=ACT, 'v'=DVE
O_EVAC = os.environ.get("KV2_OEVAC", "av")

FUSE_NORM = os.environ.get("KV2_FUSE", "1") == "1"
V_EVAC = os.environ.get("KV2_VEVAC", "dve")  # dve|mix

_CACHE = {}


def _register_recip_mul():
    """Register a fused custom-DVE op: out = approx(1/in0) * in1.

    Same BITWISE_NOT exponent-flip seed + ONE Newton-Raphson pass as
    reciprocal_approx_fast (which uses two), then multiplies by Src1 —
    six pipeline stages, one DVE instruction instead of recip + mult.
    ~0.4% worst-case reciprocal error; fine for softmax denominators."""
    from concourse import dve_ops
    from concourse.dve_spec import (
        AluOp, Bin, Spec, Src0, Src1, C0, C1, lower, _has_src1,
    )
    from concourse.dve_uop import DveOpSpec

    for op in dve_ops.OPS:
        if op.name == "RECIP1NR_MUL_ANT":
            return op

    def _ref(in0, in1, s0, s1, imm2):
        not_x = (~in0.view(np.int32)).view(np.float32)
        y0 = not_x * s0
        y1 = y0 * (s1 - in0 * y0)
        return (y1 * in1).astype(np.float32)

    _not_x = Bin(AluOp.BITWISE_NOT, Src0, Src0)
    _y0 = _not_x * C0
    _y1 = _y0 * (C1 - Src0 * _y0)
    op = dve_ops.DveOp(
        "RECIP1NR_MUL_ANT",
        Spec(body=_y1 * Src1, reference=_ref),
        subdim=False,
        uops_sha={},
    )
    dve_ops.OPS.append(op)
    dve_ops.CUSTOM_DVE_SPECS[op.name] = op.spec
    dve_ops._SUB_OPCODE_FOR_NAME[op.name] = (
        max(dve_ops._SUB_OPCODE_FOR_NAME.values()) + 1)
    assert dve_ops._SUB_OPCODE_FOR_NAME[op.name] < 0x20
    # pin the lowered-ucode sha so DveOp.compile's drift check passes
    for ver in ("v3", "v4"):
        try:
            s = DveOpSpec(
                name=op.name,
                opcode=dve_ops.get_dve_sub_opcode(op.name),
                uops=lower(op.spec, ver=ver),
                rd1_en=_has_src1(op.spec),
            )
            op.uops_sha[ver] = s.sha(ver)
        except Exception:
            pass
    return op


def _rel_bias_np(position):
    coords = np.stack(
        np.meshgrid(np.arange(P), np.arange(P), indexing="ij"), -1
    ).reshape(P * P, 2)
    rel = coords[None, :, :] - coords[:, None, :] + P
    return position[:, rel[..., 0], rel[..., 1]]  # [heads, i(query), j(key)]


def _build():
    import concourse.bass as bass  # noqa: F401
    import concourse.tile as tile
    from concourse import bacc, mybir

    f32 = mybir.dt.float32
    bf16 = mybir.dt.bfloat16
    MULT = mybir.AluOpType.mult
    DIV = mybir.AluOpType.divide
    EXP = mybir.ActivationFunctionType.Exp
    IDENT = mybir.ActivationFunctionType.Identity

    recip_mul_op = _register_recip_mul()
    nc = bacc.Bacc("TRN2", target_bir_lowering=False, debug=False,
                   num_devices=NCORES)

    x_d = nc.dram_tensor("x_sh", [B_LOC, C, HW], bf16,
                         kind="ExternalInput").ap()
    # all bf16 constants packed into one [128, 6272] tensor (one DMA):
    # cols 0:1024 wqk(2x512) | 1024:1536 wv(2x256) | 1536:2048 wo(2x256)
    # | 2048:6144 ebias(8x512) | 6144:6272 ones_blk
    cpk_d = nc.dram_tensor("consts_pk", [128, 6272], bf16,
                           kind="ExternalInput").ap()
    bout_d = nc.dram_tensor("b_out2", [2, 128, 1], f32, kind="ExternalInput").ap()
    y_d = nc.dram_tensor("y_sh", [B_LOC, C, HW], bf16,
                         kind="ExternalOutput").ap()

    from contextlib import ExitStack
    with tile.TileContext(nc) as tc:
        with ExitStack() as stack:
            constp = stack.enter_context(tc.tile_pool(name="const", bufs=1))
            xinp = stack.enter_context(tc.tile_pool(name="xin", bufs=2))
            xwmp = stack.enter_context(tc.tile_pool(name="xwm", bufs=32))
            qkp = stack.enter_context(tc.tile_pool(name="qkwm", bufs=8))
            vtp = stack.enter_context(tc.tile_pool(name="vt", bufs=2))
            attp = stack.enter_context(tc.tile_pool(name="at", bufs=int(os.environ.get("KV2_ATB", "5"))))
            at2p = stack.enter_context(tc.tile_pool(name="at2", bufs=int(os.environ.get("KV2_AT2B", "5"))))
            rrp = (None if FUSE_NORM else
                   stack.enter_context(tc.tile_pool(name="rr", bufs=2)))
            osbp = stack.enter_context(tc.tile_pool(name="osb", bufs=4))
            ytp = stack.enter_context(tc.tile_pool(name="yt", bufs=4))
            pprojp = stack.enter_context(tc.tile_pool(
                name="pproj", bufs=1 if MERGE_SD else PPROJ_BUFS,
                space="PSUM"))
            if MERGE_SD:
                psdp = stack.enter_context(tc.tile_pool(
                    name="psd", bufs=3, space="PSUM"))
                pscorep = pdenp = psdp
            else:
                pscorep = stack.enter_context(tc.tile_pool(
                    name="pscore", bufs=PSCORE_BUFS, space="PSUM"))
                pdenp = stack.enter_context(tc.tile_pool(
                    name="pden", bufs=PDEN_BUFS, space="PSUM"))
                psdp = None
            pobankp = stack.enter_context(tc.tile_pool(
                name="pobank", bufs=POBANK_BUFS, space="PSUM"))
            # ---- constants (weights DMA'd first; ebias+ones can land
            # later — they are first read in phase D) ----
            cpk = constp.tile([128, 6272], bf16, tag="cpk", name="cpk")
            nc.sync.dma_start(cpk[:, :2048], cpk_d[:, :2048])
            wqk_sb = [cpk[:, kc * 512:(kc + 1) * 512] for kc in range(2)]
            wv_sb = [cpk[:, 1024 + kc * 256:1024 + (kc + 1) * 256]
                     for kc in range(2)]
            wo_sb = [cpk[:, 1536 + kc * 256:1536 + (kc + 1) * 256]
                     for kc in range(2)]
            ebias_sb = [cpk[:, 2048 + h * 512:2048 + (h + 1) * 512]
                        for h in range(HEADS)]
            ones_sb = cpk[:, 6144:6272]
            bo = constp.tile([128, 2], f32, tag="bo", name="bo")
            nc.sync.dma_start(
                bo[:].rearrange("p (kc one) -> p kc one", kc=2, one=1),
                bout_d[:, :, :].rearrange("kc p one -> p kc one"))
            bout_sb = [bo[:, kc:kc + 1] for kc in range(2)]
            # warm the ACT function table while the first x DMAs run
            warm = constp.tile([128, 1], f32, tag="warm", name="warm")
            nc.scalar.copy(warm[:], bout_sb[0][:])

            def phase_A(b, after_first=None):
                """x load + scatter; returns x_wm[cc][wr] per-window-row
                tiles [128, 512] so consumers unblock as rows arrive.
                after_first is emitted right after the first DMA so small
                const loads don't delay the first x transfer (or vice
                versa)."""
                x_wm = [[xwmp.tile([128, 512], bf16, tag="xwm", name="xwm")
                         for _ in range(NW)] for _ in range(2)]
                first = [True]
                for t2 in range(2):
                    for cc in range(2):  # cc inner: both chunks arrive early
                        xi = xinp.tile([128, 2048], bf16, tag="xin",
                                       name="xin")
                        base = x_d[b, cc * 128:(cc + 1) * 128,
                                   t2 * 2048:(t2 + 1) * 2048]
                        if b == 0 and t2 == 0:
                            # image 0 startup: land window-row 0 first so
                            # the scatter + projections begin ~1.5us sooner
                            nc.sync.dma_start(xi[:, :512], base[:, :512])
                            nc.sync.dma_start(xi[:, 512:], base[:, 512:])
                        else:
                            nc.sync.dma_start(xi[:], base)
                        if first[0]:
                            first[0] = False
                            if after_first:
                                after_first()
                        for u in range(4):
                            wr = 4 * t2 + u
                            src = xi[:, u * 512:(u + 1) * 512].rearrange(
                                "p (ph ww pw) -> p ph ww pw", ph=P, ww=NW, pw=P
                            )
                            dst = x_wm[cc][wr][:].rearrange(
                                "p (ww ph pw) -> p ph ww pw", ww=NW, ph=P, pw=P
                            )
                            # image 0's scatter paces startup: split it
                            # across Pool and DVE; image 1 overlaps compute
                            if b == 0 and u % 2 == 1:
                                nc.vector.tensor_copy(dst, src)
                            else:
                                nc.gpsimd.tensor_copy(dst, src)
                return x_wm

            def make_BC(x_wm):
                """Thunks for q/k/v projection units (B and C interleaved)."""
                q_wm = [qkp.tile([128, HW], bf16, tag="qkwm", name="qwm")
                        for _ in range(2)]
                k_wm = [qkp.tile([128, HW], bf16, tag="qkwm", name="kwm")
                        for _ in range(2)]
                vt = vtp.tile([128, 32 * 256], bf16, tag="vt", name="vt")
                dests = [q_wm[0], q_wm[1], k_wm[0], k_wm[1]]

                def b_unit(mc, nt):
                    def run():
                        ps = pprojp.tile([128, 512], f32, tag="pp", name="pp")
                        for kc in range(2):
                            nc.tensor.matmul(
                                ps[:],
                                lhsT=wqk_sb[kc][:, mc * 128:(mc + 1) * 128],
                                rhs=x_wm[kc][nt][:],
                                start=(kc == 0), stop=(kc == 1),
                            )
                        dst = dests[mc][:, nt * 512:(nt + 1) * 512]
                        if QK_EVAC[mc] == 'a':
                            nc.scalar.copy(dst, ps[:])
                        else:
                            nc.vector.tensor_copy(dst, ps[:])
                    return run

                def c_unit(pe):
                    def run():
                        ps = pprojp.tile([128, 512], f32, tag="pp", name="pp")
                        for pp_ in range(2):
                            p = pe + pp_
                            wr, wc = p // 4, p % 4
                            for kc in range(2):
                                nc.tensor.matmul(
                                    ps[:, pp_ * 256:(pp_ + 1) * 256],
                                    lhsT=x_wm[kc][wr][:, wc * 128:
                                                      (wc + 1) * 128],
                                    rhs=wv_sb[kc][:],
                                    start=(kc == 0), stop=(kc == 1),
                                )
                        dst = vt[:, pe * 256:(pe + 2) * 256]
                        if V_EVAC == "mix" and (pe // 2) % 2:
                            nc.scalar.copy(dst, ps[:])
                        else:
                            nc.vector.tensor_copy(dst, ps[:])
                    return run

                units = []
                # nt-major so the window-rows phase D needs first are
                # produced first; C pairs of the matching window rows ride
                # along (pairs 4nt..4nt+3 live in window-row nt)
                cs = [c_unit(pe) for pe in range(0, 32, 2)]
                bs = [b_unit(mc, nt) for nt in range(8) for mc in range(4)]
                ci = iter(cs)
                for i, bu in enumerate(bs):
                    units.append(bu)
                    if i % 2 == 1:
                        nxt = next(ci, None)
                        if nxt:
                            units.append(nxt)
                units.extend(ci)
                return q_wm, k_wm, vt, units

            def phase_D(qkv, fillers, e_factory=None):
                """Attention for one image; pulls a filler thunk between
                pipeline steps. e_factory(o_sb) returns 16 output-projection
                thunks for THIS image (mc-major); the 4 units of each
                16-window block are emitted right after that block's o
                evacuation."""
                q_wm, k_wm, vt = qkv
                o_sb = [osbp.tile([128, HW], bf16, tag="osb", name="osb")
                        for _ in range(2)]
                e_units = e_factory(o_sb) if e_factory else None
                fit = iter(fillers)

                step_no = [0]

                def fill():
                    n = FILL_PER_STEP
                    if FILL_RAMP and step_no[0] >= FILL_RAMP:
                        n += 1
                    step_no[0] += 1
                    for _ in range(n):
                        f = next(fit, None)
                        if f:
                            f()

                NU = 4 * HEADS  # global units u = blk*8 + h
                obanks = [{} for _ in range(4)]
                at_t = [None] * NU
                at2_t = [None] * NU

                def get_obank(blk, hc, par):
                    # lazy alloc: first o matmul of the block allocates,
                    # giving the previous block's evacs time to free bufs
                    ob = obanks[blk]
                    if (hc, par) not in ob:
                        ob[(hc, par)] = pobankp.tile(
                            [128, 512], f32, tag="ob", name="ob")
                    return ob[(hc, par)]

                def do_scores(u):
                    blk, h = u // 8, u % 8
                    hc, hr = h // 4, h % 4
                    ps_s = pscorep.tile([128, 512], f32, tag="sc", name="sc")
                    for wl in range(16):
                        w = blk * 16 + wl
                        par, pl = w % 2, wl // 2
                        nc.tensor.matmul(
                            ps_s[par * 64:(par + 1) * 64,
                                 pl * 64:(pl + 1) * 64],
                            lhsT=k_wm[hc][hr * 32:(hr + 1) * 32,
                                          w * 64:(w + 1) * 64],
                            rhs=q_wm[hc][hr * 32:(hr + 1) * 32,
                                         w * 64:(w + 1) * 64],
                            start=True, stop=True,
                            tile_position=(hr * 32, par * 64),
                        )
                    at = attp.tile([128, 512], bf16, tag="at", name="at")
                    nc.scalar.activation(at[:], ps_s[:], EXP)
                    if EBIAS_ENG == "pool":
                        nc.gpsimd.tensor_tensor(
                            at[:], at[:], ebias_sb[h], MULT)
                    else:
                        nc.vector.tensor_tensor(
                            at[:], at[:], ebias_sb[h], MULT)
                    at_t[u] = at

                def do_denom(u):
                    h = u % 8
                    ps_d = pdenp.tile([128, 512], f32, tag="dn", name="dn")
                    nc.tensor.matmul(ps_d[:], lhsT=ones_sb[:],
                                     rhs=at_t[u][:], start=True, stop=True)
                    at2 = at2p.tile([128, 512], bf16, tag="at2", name="at2")
                    if FUSE_NORM:
                        from concourse.dve_ops import RECIP_APPROX_FAST_CONSTS
                        c = RECIP_APPROX_FAST_CONSTS
                        nc.vector._custom_dve(
                            recip_mul_op, out=at2[:], in0=ps_d[:],
                            in1=at_t[u][:], s0=c["s0"], s1=c["s1"])
                    else:
                        rr = rrp.tile([128, 512], f32, tag="rr", name="rr")
                        nc.vector.reciprocal_approx_fast(rr[:], ps_d[:])
                        eng = NORM_ENG if NORM_ENG != "split" else (
                            "pool" if h % 2 else "dve")
                        if eng == "pool":
                            nc.gpsimd.tensor_tensor(at2[:], at_t[u][:],
                                                    rr[:], MULT)
                        else:
                            nc.vector.tensor_tensor(at2[:], at_t[u][:],
                                                    rr[:], MULT)
                    at2_t[u] = at2
                    at_t[u] = None

                def do_o(u):
                    blk, h = u // 8, u % 8
                    hc, hr = h // 4, h % 4
                    for wl in range(16):
                        w = blk * 16 + wl
                        par, pl = w % 2, wl // 2
                        pglob = blk * 8 + pl
                        nc.tensor.matmul(
                            get_obank(blk, hc, par)[hr * 32:(hr + 1) * 32,
                                                    pl * 64:(pl + 1) * 64],
                            lhsT=vt[par * 64:(par + 1) * 64,
                                    pglob * 256 + h * 32:
                                    pglob * 256 + (h + 1) * 32],
                            rhs=at2_t[u][par * 64:(par + 1) * 64,
                                         pl * 64:(pl + 1) * 64],
                            start=True, stop=True,
                            tile_position=(par * 64, hr * 32),
                        )
                    at2_t[u] = None
                    if u % 8 == 7:
                        finish_blk(blk)

                def finish_blk(blk):
                    for hc in range(2):
                        for par in range(2):
                            dst = o_sb[hc][:, blk * 1024:(blk + 1) * 1024]
                            dv = dst.rearrange("p (pl par i) -> p par pl i",
                                               pl=8, par=2, i=64)[:, par]
                            if O_EVAC[par] == 'v':
                                nc.vector.tensor_copy(
                                    dv, obanks[blk][(hc, par)][:])
                            else:
                                nc.scalar.copy(dv, obanks[blk][(hc, par)][:])
                    if e_units is not None:
                        for mc in range(2):
                            for ng in range(2):
                                e_units[mc * 8 + 2 * blk + ng]()

                for step in range(NU + LAG_D):
                    if step < NU:
                        do_scores(step)
                    if FILL_POS == 1:
                        fill()
                    if LAG_D - 1 <= step < NU + LAG_D - 1:
                        do_denom(step - (LAG_D - 1))
                    if FILL_POS == 2:
                        fill()
                    if step >= LAG_D:
                        do_o(step - LAG_D)
                    if FILL_POS == 0:
                        fill()
                # drain leftover fillers
                for f in fit:
                    f()
                return o_sb

            def make_E(b, o_sb):
                """Thunks for output projection units; y DMA'd in [128,2048]
                batches of 4 nt units to amortize HWDGE issue cost."""
                yts = {}

                def e_unit(mc, nt):
                    def run():
                        ps = pprojp.tile([128, 512], f32, tag="pp", name="pp")
                        for kc in range(2):
                            nc.tensor.matmul(
                                ps[:],
                                lhsT=wo_sb[kc][:, mc * 128:(mc + 1) * 128],
                                rhs=o_sb[kc][:, nt * 512:(nt + 1) * 512],
                                start=(kc == 0), stop=(kc == 1),
                            )
                        g = nt // 4
                        if (mc, g) not in yts:
                            yts[(mc, g)] = ytp.tile([128, 2048], bf16,
                                                    tag="yt", name="yt")
                        yt = yts[(mc, g)]
                        sl = yt[:, (nt % 4) * 512:(nt % 4 + 1) * 512]
                        ydst = sl.rearrange(
                            "p (ph ww pw) -> p ww ph pw", ph=P, ww=NW, pw=P)
                        nc.scalar.activation(
                            ydst, ps[:].rearrange(
                                "p (ww ph pw) -> p ww ph pw",
                                ww=NW, ph=P, pw=P),
                            IDENT, bias=bout_sb[mc][:])
                        if g == 1:
                            # tail: store each 512-slice as soon as its
                            # epilogue lands so the final store is small
                            q0 = (nt % 4) * 512
                            ydma = y_d[b, mc * 128:(mc + 1) * 128,
                                       g * 2048 + q0:g * 2048 + q0 + 512]
                            (nc.scalar if Y_VIA_ACT else
                             nc.sync).dma_start(ydma, yt[:, q0:q0 + 512])
                        elif nt % 4 == 3:
                            ydma = y_d[b, mc * 128:(mc + 1) * 128,
                                       g * 2048:(g + 1) * 2048]
                            (nc.scalar if Y_VIA_ACT else
                             nc.sync).dma_start(ydma, yt[:])
                    return run
                return [e_unit(mc, nt) for mc in range(2) for nt in range(8)]

            # ---- emission schedule ----
            x_wm0 = phase_A(0)
            nc.sync.dma_start(cpk[:, 2048:], cpk_d[:, 2048:])
            q0, k0, vt0, bc0 = make_BC(x_wm0)
            # emit enough of B/C(0) to cover phase D block 0, then feed the
            # rest (plus all of B/C(1)) into D(0)'s step loop as fillers
            for u in bc0:
                u()
            x_wm1 = phase_A(1)
            q1, k1, vt1, bc1 = make_BC(x_wm1)
            if os.environ.get("KV2_E0FILL", "1") == "1":
                o_sb0 = phase_D((q0, k0, vt0), bc1)
                e0 = make_E(0, o_sb0)
                o_sb1 = phase_D((q1, k1, vt1), e0,
                                e_factory=lambda o: make_E(1, o))
            else:
                o_sb0 = phase_D((q0, k0, vt0), bc1,
                                e_factory=lambda o: make_E(0, o))
                o_sb1 = phase_D((q1, k1, vt1), [],
                                e_factory=lambda o: make_E(1, o))

    nc.compile()
    return nc


def _prep_consts(w_proj, position, w_out, b_out):
    import ml_dtypes
    bf16 = ml_dtypes.bfloat16
    scale = 1.0 / np.sqrt(np.float32(D))
    w_qkT = np.ascontiguousarray(w_proj[:512].T).astype(np.float32)
    w_qkT[:, :256] *= scale
    w_qkT = w_qkT.astype(bf16)
    w_vT = np.ascontiguousarray(w_proj[512:].T).astype(bf16)
    w_outT = np.ascontiguousarray(w_out.T).astype(bf16)
    bias = _rel_bias_np(np.asarray(position, np.float32))  # [h, i, j]
    eb = np.exp(bias).astype(np.float32)
    # ebias[h][rows j | j, cols 8 x (64 i)] = exp(bias[h, i, j])
    ebt = np.transpose(eb, (0, 2, 1))  # [h, j, i]
    ebias = np.empty((HEADS, 128, 512), np.float32)
    for h in range(HEADS):
        ebias[h] = np.tile(ebt[h], (2, 8))
    ebias = ebias.astype(bf16)
    ones_blk = np.zeros((128, 128), np.float32)
    ones_blk[:64, :64] = 1.0
    ones_blk[64:, 64:] = 1.0
    ones_blk = ones_blk.astype(bf16)
    # pack all bf16 consts: wqk | wv | wo | ebias | ones  (see _build)
    cpk = np.empty((128, 6272), bf16)
    cpk[:, 0:512] = w_qkT[:128]
    cpk[:, 512:1024] = w_qkT[128:]
    cpk[:, 1024:1280] = w_vT[:128]
    cpk[:, 1280:1536] = w_vT[128:]
    cpk[:, 1536:1792] = w_outT[:128]
    cpk[:, 1792:2048] = w_outT[128:]
    for h in range(HEADS):
        cpk[:, 2048 + h * 512:2048 + (h + 1) * 512] = ebias[h]
    cpk[:, 6144:6272] = ones_blk
    b_out2 = np.ascontiguousarray(
        np.asarray(b_out, np.float32).reshape(2, 128, 1))
    return {
        "consts_pk": cpk,
        "b_out2": b_out2,
    }


def kernel(x, w_proj, position, w_out, b_out):
    from concourse.bass_utils import run_bass_kernel_spmd

    if "nc" not in _CACHE:
        _CACHE["nc"] = _build()
    nc = _CACHE["nc"]

    import ml_dtypes
    consts = _prep_consts(w_proj, position, w_out, b_out)
    x = np.asarray(x, np.float32).reshape(B, C, HW).astype(ml_dtypes.bfloat16)
    in_maps = []
    for i in range(NCORES):
        m = dict(consts)
        m["x_sh"] = np.ascontiguousarray(x[i * B_LOC:(i + 1) * B_LOC])
        in_maps.append(m)

    res = run_bass_kernel_spmd(nc, in_maps, core_ids=list(range(NCORES)))
    out = np.concatenate([res.results[i]["y_sh"] for i in range(NCORES)],
                         axis=0)
    return out.reshape(B, C, H, W).astype(np.float32)



# revision 31
# speedup vs baseline: 1.0203x; 1.0025x over previous
"""Trainium2 Bass kernel for LocalSelfAttention2d — v2 (engine-balanced).

Full inputs in, full outputs out. Data-parallel over batch B=16 across 8
NeuronCores (2 images per core). Weights/position table replicated.

Design (per image; spatial layouts window-major m = win*64+ph*8+pw):
  A. x raster f32 DMA'd in (chunk-interleaved); GPSIMD scatter-cast -> x_wm
     bf16 window-major (Pool has no PSUM port, so it gets the SBUF-only job).
  B. q/k projection, w stationary, x_wm moving -> PSUM win-major; fat evac.
  C. v projection transposed: x_wm window-pair block stationary, w_v moving
     -> vt[j(2 windows), 256 vchan]; fat evac.
  D. attention over 32 global (head, 16-window-block) units, software-
     pipelined with lag 3: per step scores(u), filler, denom(u-2), o(u-3).
     exp (ACT), *exp(bias) (DVE), and the fused recip*mult custom DVE op
     hide behind PE work of later units. B/C units of the next image and
     E units of the previous image are the fillers.
  E. output projection; epilogue on ACT (Identity + per-channel bias +
     win->raster scatter); y stores batched [128,2048] and issued from SP
     (idle once all x loads are done; the last group splits in half so
     its first store starts two units earlier).

Engine busy per core (cost model): PE ~123.5us (82% duty), DVE ~107us,
ACT ~99us, Pool ~20us; 149.8us total vs 510.6us baseline.
"""
import os
import numpy as np

B, C, H, W = 16, 256, 64, 64
P, HEADS, D = 8, 8, 32
NCORES = 8
B_LOC = B // NCORES  # 2
HW = H * W  # 4096
NW = 8  # windows per row/col

PPROJ_BUFS = int(os.environ.get("KV2_PPROJ", "3"))
PSCORE_BUFS = int(os.environ.get("KV2_PSCORE", "2"))
PDEN_BUFS = int(os.environ.get("KV2_PDEN", "1"))
MERGE_SD = os.environ.get("KV2_MERGESD", "0") == "1"  # sc+dn share a 3-ring
FILL_POS = int(os.environ.get("KV2_FILLPOS", "1"))  # 0=end 1=mid 2=predo
NORM_ENG = os.environ.get("KV2_NORM", "dve")  # pool|dve
POBANK_BUFS = int(os.environ.get("KV2_POBANK", "2"))
Y_VIA_ACT = os.environ.get("KV2_YACT", "0") == "1"
EBIAS_ENG = os.environ.get("KV2_EBIAS", "dve")  # dve|pool
FILL_PER_STEP = int(os.environ.get("KV2_FILL", "1"))
FILL_RAMP = int(os.environ.get("KV2_FILLRAMP", "0"))  # 0=off else ramp step
LAG_D = int(os.environ.get("KV2_LAG", "3"))
# qk evac engine per mc chunk (q0,q1,k0,k1): 'a'=ACT, 'v'=DVE
QK_EVAC = os.environ.get("KV2_QKEVAC", "aava")
# o-bank evac engines for (par0, par1): 'a'=ACT, 'v'=DVE
O_EVAC = os.environ.get("KV2_OEVAC", "av")

FUSE_NORM = os.environ.get("KV2_FUSE", "1") == "1"
V_EVAC = os.environ.get("KV2_VEVAC", "dve")  # dve|mix

_CACHE = {}


def _register_recip_mul():
    """Register a fused custom-DVE op: out = approx(1/in0) * in1.

    Same BITWISE_NOT exponent-flip seed + ONE Newton-Raphson pass as
    reciprocal_approx_fast (which uses two), then multiplies by Src1 —
    six pipeline stages, one DVE instruction instead of recip + mult.
    ~0.4% worst-case reciprocal error; fine for softmax denominators."""
    from concourse import dve_ops
    from concourse.dve_spec import (
        AluOp, Bin, Spec, Src0, Src1, C0, C1, lower, _has_src1,
    )
    from concourse.dve_uop import DveOpSpec

    for op in dve_ops.OPS:
        if op.name == "RECIP1NR_MUL_ANT":
            return op

    def _ref(in0, in1, s0, s1, imm2):
        not_x = (~in0.view(np.int32)).view(np.float32)
        y0 = not_x * s0
        y1 = y0 * (s1 - in0 * y0)
        return (y1 * in1).astype(np.float32)

    _not_x = Bin(AluOp.BITWISE_NOT, Src0, Src0)
    _y0 = _not_x * C0
    _y1 = _y0 * (C1 - Src0 * _y0)
    op = dve_ops.DveOp(
        "RECIP1NR_MUL_ANT",
        Spec(body=_y1 * Src1, reference=_ref),
        subdim=False,
        uops_sha={},
    )
    dve_ops.OPS.append(op)
    dve_ops.CUSTOM_DVE_SPECS[op.name] = op.spec
    dve_ops._SUB_OPCODE_FOR_NAME[op.name] = (
        max(dve_ops._SUB_OPCODE_FOR_NAME.values()) + 1)
    assert dve_ops._SUB_OPCODE_FOR_NAME[op.name] < 0x20
    # pin the lowered-ucode sha so DveOp.compile's drift check passes
    for ver in ("v3", "v4"):
        try:
            s = DveOpSpec(
                name=op.name,
                opcode=dve_ops.get_dve_sub_opcode(op.name),
                uops=lower(op.spec, ver=ver),
                rd1_en=_has_src1(op.spec),
            )
            op.uops_sha[ver] = s.sha(ver)
        except Exception:
            pass
    return op


def _rel_bias_np(position):
    coords = np.stack(
        np.meshgrid(np.arange(P), np.arange(P), indexing="ij"), -1
    ).reshape(P * P, 2)
    rel = coords[None, :, :] - coords[:, None, :] + P
    return position[:, rel[..., 0], rel[..., 1]]  # [heads, i(query), j(key)]


def _build():
    import concourse.bass as bass  # noqa: F401
    import concourse.tile as tile
    from concourse import bacc, mybir

    f32 = mybir.dt.float32
    bf16 = mybir.dt.bfloat16
    MULT = mybir.AluOpType.mult
    DIV = mybir.AluOpType.divide
    EXP = mybir.ActivationFunctionType.Exp
    IDENT = mybir.ActivationFunctionType.Identity

    recip_mul_op = _register_recip_mul()
    nc = bacc.Bacc("TRN2", target_bir_lowering=False, debug=False,
                   num_devices=NCORES)

    x_d = nc.dram_tensor("x_sh", [B_LOC, C, HW], bf16,
                         kind="ExternalInput").ap()
    # all bf16 constants packed into one [128, 6272] tensor (one DMA):
    # cols 0:1024 wqk(2x512) | 1024:1536 wv(2x256) | 1536:2048 wo(2x256)
    # | 2048:6144 ebias(8x512) | 6144:6272 ones_blk
    cpk_d = nc.dram_tensor("consts_pk", [128, 6272], bf16,
                           kind="ExternalInput").ap()
    bout_d = nc.dram_tensor("b_out2", [2, 128, 1], f32, kind="ExternalInput").ap()
    y_d = nc.dram_tensor("y_sh", [B_LOC, C, HW], bf16,
                         kind="ExternalOutput").ap()

    from contextlib import ExitStack
    with tile.TileContext(nc) as tc:
        with ExitStack() as stack:
            constp = stack.enter_context(tc.tile_pool(name="const", bufs=1))
            xinp = stack.enter_context(tc.tile_pool(name="xin", bufs=2))
            xwmp = stack.enter_context(tc.tile_pool(name="xwm", bufs=32))
            qkp = stack.enter_context(tc.tile_pool(name="qkwm", bufs=8))
            vtp = stack.enter_context(tc.tile_pool(name="vt", bufs=2))
            attp = stack.enter_context(tc.tile_pool(name="at", bufs=int(os.environ.get("KV2_ATB", "5"))))
            at2p = stack.enter_context(tc.tile_pool(name="at2", bufs=int(os.environ.get("KV2_AT2B", "5"))))
            rrp = (None if FUSE_NORM else
                   stack.enter_context(tc.tile_pool(name="rr", bufs=2)))
            osbp = stack.enter_context(tc.tile_pool(name="osb", bufs=4))
            ytp = stack.enter_context(tc.tile_pool(name="yt", bufs=4))
            pprojp = stack.enter_context(tc.tile_pool(
                name="pproj", bufs=1 if MERGE_SD else PPROJ_BUFS,
                space="PSUM"))
            if MERGE_SD:
                psdp = stack.enter_context(tc.tile_pool(
                    name="psd", bufs=3, space="PSUM"))
                pscorep = pdenp = psdp
            else:
                pscorep = stack.enter_context(tc.tile_pool(
                    name="pscore", bufs=PSCORE_BUFS, space="PSUM"))
                pdenp = stack.enter_context(tc.tile_pool(
                    name="pden", bufs=PDEN_BUFS, space="PSUM"))
                psdp = None
            pobankp = stack.enter_context(tc.tile_pool(
                name="pobank", bufs=POBANK_BUFS, space="PSUM"))
            # ---- constants (weights DMA'd first; ebias+ones can land
            # later — they are first read in phase D) ----
            cpk = constp.tile([128, 6272], bf16, tag="cpk", name="cpk")
            nc.sync.dma_start(cpk[:, :2048], cpk_d[:, :2048])
            wqk_sb = [cpk[:, kc * 512:(kc + 1) * 512] for kc in range(2)]
            wv_sb = [cpk[:, 1024 + kc * 256:1024 + (kc + 1) * 256]
                     for kc in range(2)]
            wo_sb = [cpk[:, 1536 + kc * 256:1536 + (kc + 1) * 256]
                     for kc in range(2)]
            ebias_sb = [cpk[:, 2048 + h * 512:2048 + (h + 1) * 512]
                        for h in range(HEADS)]
            ones_sb = cpk[:, 6144:6272]
            bo = constp.tile([128, 2], f32, tag="bo", name="bo")
            nc.sync.dma_start(
                bo[:].rearrange("p (kc one) -> p kc one", kc=2, one=1),
                bout_d[:, :, :].rearrange("kc p one -> p kc one"))
            bout_sb = [bo[:, kc:kc + 1] for kc in range(2)]
            # warm the ACT function table while the first x DMAs run
            warm = constp.tile([128, 1], f32, tag="warm", name="warm")
            nc.scalar.copy(warm[:], bout_sb[0][:])

            def phase_A(b, after_first=None):
                """x load + scatter; returns x_wm[cc][wr] per-window-row
                tiles [128, 512] so consumers unblock as rows arrive.
                after_first is emitted right after the first DMA so small
                const loads don't delay the first x transfer (or vice
                versa)."""
                x_wm = [[xwmp.tile([128, 512], bf16, tag="xwm", name="xwm")
                         for _ in range(NW)] for _ in range(2)]
                first = [True]
                for t2 in range(2):
                    for cc in range(2):  # cc inner: both chunks arrive early
                        xi = xinp.tile([128, 2048], bf16, tag="xin",
                                       name="xin")
                        base = x_d[b, cc * 128:(cc + 1) * 128,
                                   t2 * 2048:(t2 + 1) * 2048]
                        if b == 0 and t2 == 0:
                            # image 0 startup: land window-row 0 first so
                            # the scatter + projections begin ~1.5us sooner
                            nc.sync.dma_start(xi[:, :512], base[:, :512])
                            nc.sync.dma_start(xi[:, 512:], base[:, 512:])
                        else:
                            nc.sync.dma_start(xi[:], base)
                        if first[0]:
                            first[0] = False
                            if after_first:
                                after_first()
                        for u in range(4):
                            wr = 4 * t2 + u
                            src = xi[:, u * 512:(u + 1) * 512].rearrange(
                                "p (ph ww pw) -> p ph ww pw", ph=P, ww=NW, pw=P
                            )
                            dst = x_wm[cc][wr][:].rearrange(
                                "p (ww ph pw) -> p ph ww pw", ww=NW, ph=P, pw=P
                            )
                            # image 0's scatter paces startup: split it
                            # across Pool and DVE; image 1 overlaps compute
                            if b == 0 and u % 2 == 1:
                                nc.vector.tensor_copy(dst, src)
                            else:
                                nc.gpsimd.tensor_copy(dst, src)
                return x_wm

            def make_BC(x_wm):
                """Thunks for q/k/v projection units (B and C interleaved)."""
                q_wm = [qkp.tile([128, HW], bf16, tag="qkwm", name="qwm")
                        for _ in range(2)]
                k_wm = [qkp.tile([128, HW], bf16, tag="qkwm", name="kwm")
                        for _ in range(2)]
                vt = vtp.tile([128, 32 * 256], bf16, tag="vt", name="vt")
                dests = [q_wm[0], q_wm[1], k_wm[0], k_wm[1]]

                def b_unit(mc, nt):
                    def run():
                        ps = pprojp.tile([128, 512], f32, tag="pp", name="pp")
                        for kc in range(2):
                            nc.tensor.matmul(
                                ps[:],
                                lhsT=wqk_sb[kc][:, mc * 128:(mc + 1) * 128],
                                rhs=x_wm[kc][nt][:],
                                start=(kc == 0), stop=(kc == 1),
                            )
                        dst = dests[mc][:, nt * 512:(nt + 1) * 512]
                        if QK_EVAC[mc] == 'a':
                            nc.scalar.copy(dst, ps[:])
                        else:
                            nc.vector.tensor_copy(dst, ps[:])
                    return run

                def c_unit(pe):
                    def run():
                        ps = pprojp.tile([128, 512], f32, tag="pp", name="pp")
                        for pp_ in range(2):
                            p = pe + pp_
                            wr, wc = p // 4, p % 4
                            for kc in range(2):
                                nc.tensor.matmul(
                                    ps[:, pp_ * 256:(pp_ + 1) * 256],
                                    lhsT=x_wm[kc][wr][:, wc * 128:
                                                      (wc + 1) * 128],
                                    rhs=wv_sb[kc][:],
                                    start=(kc == 0), stop=(kc == 1),
                                )
                        dst = vt[:, pe * 256:(pe + 2) * 256]
                        if V_EVAC == "mix" and (pe // 2) % 2:
                            nc.scalar.copy(dst, ps[:])
                        else:
                            nc.vector.tensor_copy(dst, ps[:])
                    return run

                units = []
                # nt-major so the window-rows phase D needs first are
                # produced first; C pairs of the matching window rows ride
                # along (pairs 4nt..4nt+3 live in window-row nt)
                cs = [c_unit(pe) for pe in range(0, 32, 2)]
                bs = [b_unit(mc, nt) for nt in range(8) for mc in range(4)]
                ci = iter(cs)
                for i, bu in enumerate(bs):
                    units.append(bu)
                    if i % 2 == 1:
                        nxt = next(ci, None)
                        if nxt:
                            units.append(nxt)
                units.extend(ci)
                return q_wm, k_wm, vt, units

            def phase_D(qkv, fillers, e_factory=None):
                """Attention for one image; pulls a filler thunk between
                pipeline steps. e_factory(o_sb) returns 16 output-projection
                thunks for THIS image (mc-major); the 4 units of each
                16-window block are emitted right after that block's o
                evacuation."""
                q_wm, k_wm, vt = qkv
                o_sb = [osbp.tile([128, HW], bf16, tag="osb", name="osb")
                        for _ in range(2)]
                e_units = e_factory(o_sb) if e_factory else None
                fit = iter(fillers)

                step_no = [0]
                # spread sparse filler lists across the whole phase instead
                # of front-loading them 1/step (D(1) has only 16 E(0) units
                # for 35 steps; unpaced, the last ~20 steps starve)
                n_f = len(fillers)
                period = max(1, (4 * HEADS + LAG_D) // n_f) if n_f else 1

                def fill():
                    n = FILL_PER_STEP
                    if FILL_RAMP and step_no[0] >= FILL_RAMP:
                        n += 1
                    if step_no[0] % period:
                        n = 0
                    step_no[0] += 1
                    for _ in range(n):
                        f = next(fit, None)
                        if f:
                            f()

                NU = 4 * HEADS  # global units u = blk*8 + h
                obanks = [{} for _ in range(4)]
                at_t = [None] * NU
                at2_t = [None] * NU

                def get_obank(blk, hc, par):
                    # lazy alloc: first o matmul of the block allocates,
                    # giving the previous block's evacs time to free bufs
                    ob = obanks[blk]
                    if (hc, par) not in ob:
                        ob[(hc, par)] = pobankp.tile(
                            [128, 512], f32, tag="ob", name="ob")
                    return ob[(hc, par)]

                def do_scores(u):
                    blk, h = u // 8, u % 8
                    hc, hr = h // 4, h % 4
                    ps_s = pscorep.tile([128, 512], f32, tag="sc", name="sc")
                    for wl in range(16):
                        w = blk * 16 + wl
                        par, pl = w % 2, wl // 2
                        nc.tensor.matmul(
                            ps_s[par * 64:(par + 1) * 64,
                                 pl * 64:(pl + 1) * 64],
                            lhsT=k_wm[hc][hr * 32:(hr + 1) * 32,
                                          w * 64:(w + 1) * 64],
                            rhs=q_wm[hc][hr * 32:(hr + 1) * 32,
                                         w * 64:(w + 1) * 64],
                            start=True, stop=True,
                            tile_position=(hr * 32, par * 64),
                        )
                    at = attp.tile([128, 512], bf16, tag="at", name="at")
                    nc.scalar.activation(at[:], ps_s[:], EXP)
                    if EBIAS_ENG == "pool":
                        nc.gpsimd.tensor_tensor(
                            at[:], at[:], ebias_sb[h], MULT)
                    else:
                        nc.vector.tensor_tensor(
                            at[:], at[:], ebias_sb[h], MULT)
                    at_t[u] = at

                def do_denom(u):
                    h = u % 8
                    ps_d = pdenp.tile([128, 512], f32, tag="dn", name="dn")
                    nc.tensor.matmul(ps_d[:], lhsT=ones_sb[:],
                                     rhs=at_t[u][:], start=True, stop=True)
                    at2 = at2p.tile([128, 512], bf16, tag="at2", name="at2")
                    if FUSE_NORM:
                        from concourse.dve_ops import RECIP_APPROX_FAST_CONSTS
                        c = RECIP_APPROX_FAST_CONSTS
                        nc.vector._custom_dve(
                            recip_mul_op, out=at2[:], in0=ps_d[:],
                            in1=at_t[u][:], s0=c["s0"], s1=c["s1"])
                    else:
                        rr = rrp.tile([128, 512], f32, tag="rr", name="rr")
                        nc.vector.reciprocal_approx_fast(rr[:], ps_d[:])
                        eng = NORM_ENG if NORM_ENG != "split" else (
                            "pool" if h % 2 else "dve")
                        if eng == "pool":
                            nc.gpsimd.tensor_tensor(at2[:], at_t[u][:],
                                                    rr[:], MULT)
                        else:
                            nc.vector.tensor_tensor(at2[:], at_t[u][:],
                                                    rr[:], MULT)
                    at2_t[u] = at2
                    at_t[u] = None

                def do_o(u):
                    blk, h = u // 8, u % 8
                    hc, hr = h // 4, h % 4
                    for wl in range(16):
                        w = blk * 16 + wl
                        par, pl = w % 2, wl // 2
                        pglob = blk * 8 + pl
                        nc.tensor.matmul(
                            get_obank(blk, hc, par)[hr * 32:(hr + 1) * 32,
                                                    pl * 64:(pl + 1) * 64],
                            lhsT=vt[par * 64:(par + 1) * 64,
                                    pglob * 256 + h * 32:
                                    pglob * 256 + (h + 1) * 32],
                            rhs=at2_t[u][par * 64:(par + 1) * 64,
                                         pl * 64:(pl + 1) * 64],
                            start=True, stop=True,
                            tile_position=(par * 64, hr * 32),
                        )
                    at2_t[u] = None
                    if u % 8 == 7:
                        finish_blk(blk)

                def finish_blk(blk):
                    for hc in range(2):
                        for par in range(2):
                            dst = o_sb[hc][:, blk * 1024:(blk + 1) * 1024]
                            dv = dst.rearrange("p (pl par i) -> p par pl i",
                                               pl=8, par=2, i=64)[:, par]
                            if O_EVAC[par] == 'v':
                                nc.vector.tensor_copy(
                                    dv, obanks[blk][(hc, par)][:])
                            else:
                                nc.scalar.copy(dv, obanks[blk][(hc, par)][:])
                    if e_units is not None:
                        for mc in range(2):
                            for ng in range(2):
                                e_units[mc * 8 + 2 * blk + ng]()

                for step in range(NU + LAG_D):
                    if step < NU:
                        do_scores(step)
                    if FILL_POS == 1:
                        fill()
                    if LAG_D - 1 <= step < NU + LAG_D - 1:
                        do_denom(step - (LAG_D - 1))
                    if FILL_POS == 2:
                        fill()
                    if step >= LAG_D:
                        do_o(step - LAG_D)
                    if FILL_POS == 0:
                        fill()
                # drain leftover fillers
                for f in fit:
                    f()
                return o_sb

            def make_E(b, o_sb):
                """Thunks for output projection units; y DMA'd in [128,2048]
                batches of 4 nt units to amortize HWDGE issue cost."""
                yts = {}

                def e_unit(mc, nt):
                    def run():
                        ps = pprojp.tile([128, 512], f32, tag="pp", name="pp")
                        for kc in range(2):
                            nc.tensor.matmul(
                                ps[:],
                                lhsT=wo_sb[kc][:, mc * 128:(mc + 1) * 128],
                                rhs=o_sb[kc][:, nt * 512:(nt + 1) * 512],
                                start=(kc == 0), stop=(kc == 1),
                            )
                        g = nt // 4
                        if (mc, g) not in yts:
                            yts[(mc, g)] = ytp.tile([128, 2048], bf16,
                                                    tag="yt", name="yt")
                        yt = yts[(mc, g)]
                        sl = yt[:, (nt % 4) * 512:(nt % 4 + 1) * 512]
                        ydst = sl.rearrange(
                            "p (ph ww pw) -> p ww ph pw", ph=P, ww=NW, pw=P)
                        nc.scalar.activation(
                            ydst, ps[:].rearrange(
                                "p (ww ph pw) -> p ww ph pw",
                                ww=NW, ph=P, pw=P),
                            IDENT, bias=bout_sb[mc][:])
                        if g == 1:
                            # tail: store each 512-slice as soon as its
                            # epilogue lands so the final store is small
                            q0 = (nt % 4) * 512
                            ydma = y_d[b, mc * 128:(mc + 1) * 128,
                                       g * 2048 + q0:g * 2048 + q0 + 512]
                            (nc.scalar if Y_VIA_ACT else
                             nc.sync).dma_start(ydma, yt[:, q0:q0 + 512])
                        elif nt % 4 == 3:
                            ydma = y_d[b, mc * 128:(mc + 1) * 128,
                                       g * 2048:(g + 1) * 2048]
                            (nc.scalar if Y_VIA_ACT else
                             nc.sync).dma_start(ydma, yt[:])
                    return run
                return [e_unit(mc, nt) for mc in range(2) for nt in range(8)]

            # ---- emission schedule ----
            x_wm0 = phase_A(0)
            nc.sync.dma_start(cpk[:, 2048:], cpk_d[:, 2048:])
            q0, k0, vt0, bc0 = make_BC(x_wm0)
            # emit enough of B/C(0) to cover phase D block 0, then feed the
            # rest (plus all of B/C(1)) into D(0)'s step loop as fillers
            for u in bc0:
                u()
            x_wm1 = phase_A(1)
            q1, k1, vt1, bc1 = make_BC(x_wm1)
            if os.environ.get("KV2_E0FILL", "1") == "1":
                o_sb0 = phase_D((q0, k0, vt0), bc1)
                e0 = make_E(0, o_sb0)
                o_sb1 = phase_D((q1, k1, vt1), e0,
                                e_factory=lambda o: make_E(1, o))
            else:
                o_sb0 = phase_D((q0, k0, vt0), bc1,
                                e_factory=lambda o: make_E(0, o))
                o_sb1 = phase_D((q1, k1, vt1), [],
                                e_factory=lambda o: make_E(1, o))

    nc.compile()
    return nc


def _prep_consts(w_proj, position, w_out, b_out):
    import ml_dtypes
    bf16 = ml_dtypes.bfloat16
    scale = 1.0 / np.sqrt(np.float32(D))
    w_qkT = np.ascontiguousarray(w_proj[:512].T).astype(np.float32)
    w_qkT[:, :256] *= scale
    w_qkT = w_qkT.astype(bf16)
    w_vT = np.ascontiguousarray(w_proj[512:].T).astype(bf16)
    w_outT = np.ascontiguousarray(w_out.T).astype(bf16)
    bias = _rel_bias_np(np.asarray(position, np.float32))  # [h, i, j]
    eb = np.exp(bias).astype(np.float32)
    # ebias[h][rows j | j, cols 8 x (64 i)] = exp(bias[h, i, j])
    ebt = np.transpose(eb, (0, 2, 1))  # [h, j, i]
    ebias = np.empty((HEADS, 128, 512), np.float32)
    for h in range(HEADS):
        ebias[h] = np.tile(ebt[h], (2, 8))
    ebias = ebias.astype(bf16)
    ones_blk = np.zeros((128, 128), np.float32)
    ones_blk[:64, :64] = 1.0
    ones_blk[64:, 64:] = 1.0
    ones_blk = ones_blk.astype(bf16)
    # pack all bf16 consts: wqk | wv | wo | ebias | ones  (see _build)
    cpk = np.empty((128, 6272), bf16)
    cpk[:, 0:512] = w_qkT[:128]
    cpk[:, 512:1024] = w_qkT[128:]
    cpk[:, 1024:1280] = w_vT[:128]
    cpk[:, 1280:1536] = w_vT[128:]
    cpk[:, 1536:1792] = w_outT[:128]
    cpk[:, 1792:2048] = w_outT[128:]
    for h in range(HEADS):
        cpk[:, 2048 + h * 512:2048 + (h + 1) * 512] = ebias[h]
    cpk[:, 6144:6272] = ones_blk
    b_out2 = np.ascontiguousarray(
        np.asarray(b_out, np.float32).reshape(2, 128, 1))
    return {
        "consts_pk": cpk,
        "b_out2": b_out2,
    }


def kernel(x, w_proj, position, w_out, b_out):
    from concourse.bass_utils import run_bass_kernel_spmd

    if "nc" not in _CACHE:
        _CACHE["nc"] = _build()
    nc = _CACHE["nc"]

    import ml_dtypes
    consts = _prep_consts(w_proj, position, w_out, b_out)
    x = np.asarray(x, np.float32).reshape(B, C, HW).astype(ml_dtypes.bfloat16)
    in_maps = []
    for i in range(NCORES):
        m = dict(consts)
        m["x_sh"] = np.ascontiguousarray(x[i * B_LOC:(i + 1) * B_LOC])
        in_maps.append(m)

    res = run_bass_kernel_spmd(nc, in_maps, core_ids=list(range(NCORES)))
    out = np.concatenate([res.results[i]["y_sh"] for i in range(NCORES)],
                         axis=0)
    return out.reshape(B, C, H, W).astype(np.float32)

